# revision 1
# baseline (speedup 1.0000x reference)
"""ChebConv (K=4) GNN kernel for 8 Trainium2 NeuronCores.

Strategy (1D node partition, pull-mode message passing):
  - Nodes sharded 8 ways (6250/core, padded to 6272 = 49 blocks of 128).
  - Scaled states Y_k = X_k * d^-1/2 so the src-side degree scaling is folded
    into the gather table; recurrence runs on Y with d^-1 folded per dst.
  - Per Chebyshev step: AllGather Y rows -> DRAM table, dma_gather (SWDGE)
    the src rows of this core's edges, segment-sum by dst via one-hot
    matmuls on TensorE (PSUM accumulate per 128-node block), elementwise
    recurrence on VectorE.
  - One-hots built from uploaded per-tile dst values: DVE is_equal against an
    iota row (one op), a fraction on ScalarE via Abs+Relu (exact for ints).
  - Final: out = relu(d^+1/2 * (Yt^T.T @ W) + b) with 2 K=128 matmuls/tile.

The same Bass program runs SPMD on all 8 cores (shapes/budgets are global
maxima); per-core behavior differs only through input data.
"""

import math
import os
import sys

import numpy as np

sys.path.insert(0, "/opt/trn_rl_repo")

import concourse.bacc as bacc  # noqa: E402
import concourse.bass as bass  # noqa: E402
import concourse.mybir as mybir  # noqa: E402
import concourse.tile as tile  # noqa: E402
from concourse.bass_utils import run_bass_kernel_spmd  # noqa: E402

P = 128
N_CORES = 8
F_IN = 64
K_CHEB = 4
F_OUT = 256
FP32 = mybir.dt.float32
I16 = mybir.dt.int16

# fraction of one-hot tiles built on ScalarE (ACT) instead of VectorE
ACT_FRAC = 0.30


# ---------------------------------------------------------------------------
# host-side graph preprocessing (indices only + trivial degree vectors)
# ---------------------------------------------------------------------------
def preprocess(signal, src, dst, lambda_max, W, b):
    n_nodes = signal.shape[0]
    n_shard = (n_nodes + N_CORES - 1) // N_CORES          # 6250
    nb = (n_shard + P - 1) // P                           # 49 blocks/core
    ncols = nb * P                                        # 6272
    tab_rows = N_CORES * ncols                            # 50176
    half_rows = tab_rows // 2                             # 25088
    assert half_rows - 1 <= 32767, "int16 gather index range exceeded"

    deg = np.bincount(dst, minlength=n_nodes).astype(np.float64)
    degc = np.maximum(deg, 1.0)
    dsqrt = degc ** -0.5
    ds2 = 1.0 / degc
    idsq = degc ** 0.5

    re = 2.0 / float(np.asarray(lambda_max).reshape(-1)[0])
    c1 = re - 1.0
    c2 = 2.0 * (re - 1.0)

    owner = dst // n_shard
    local = dst - owner * n_shard
    blk = local // P
    # table row for a global node id (p-major within its shard)
    r = src % n_shard
    tab_row = (src // n_shard) * ncols + (r % P) * nb + (r // P)
    halfid = (tab_row >= half_rows).astype(np.int64)

    # group edges by (core, block, half)
    key = (owner * nb + blk) * 2 + halfid
    order = np.argsort(key, kind="stable")
    key_s = key[order]
    tab_s = tab_row[order]
    dloc_s = (local - blk * P)[order]

    counts = np.bincount(key, minlength=N_CORES * nb * 2).reshape(N_CORES, nb, 2)
    t_h = [max(1, int(math.ceil(counts[:, :, h].max() / P))) for h in range(2)]
    T0, T1 = t_h
    nt_blk = T0 + T1                      # matmul tiles per block
    nt = nb * nt_blk                      # matmul tiles per core per iteration

    # chunking of gather calls: CH blocks per call (last chunk may be short)
    ch = max(1, min(nb, 27 // max(T0, T1)))
    chunk_plan = [(s, min(ch, nb - s)) for s in range(0, nb, ch)]

    starts = np.zeros(N_CORES * nb * 2 + 1, dtype=np.int64)
    np.cumsum(np.bincount(key_s, minlength=N_CORES * nb * 2), out=starts[1:])

    idx_arrs = [[], []]   # per half: [core] -> int16 [128, nb*T_h*8]
    dstv_arrs = []        # per core: f32 [128, nt]
    for c in range(N_CORES):
        dv = np.full((nb, nt_blk, P), -1.0, dtype=np.float32)
        for h, T in ((0, T0), (1, T1)):
            ids = np.zeros((nb, T * P), dtype=np.int16)
            for bk in range(nb):
                kk = (c * nb + bk) * 2 + h
                s, e = starts[kk], starts[kk + 1]
                cnt = e - s
                ids[bk, :cnt] = (tab_s[s:e] - h * half_rows).astype(np.int16)
                jofs = 0 if h == 0 else T0
                dv[bk, jofs:jofs + T, :].reshape(-1)[:cnt] = dloc_s[s:e]
            flat = ids.reshape(-1)                       # [nb*T*P]
            wrap = flat.reshape(-1, 16).T.copy()         # [16, L/16]
            idx_arrs[h].append(np.tile(wrap, (8, 1)))    # [128, L/16]
        dstv_arrs.append(
            dv.reshape(nb * nt_blk, P).T.copy()          # [128, nt]
        )

    # per-core node-indexed aux arrays (value at [p, t] -> node t*128+p)
    def cols_of(vec, fill):
        out = np.full((N_CORES, ncols), fill, dtype=np.float32)
        out[:, :n_shard] = vec.reshape(N_CORES, n_shard)
        return out.reshape(N_CORES, nb, P).transpose(0, 2, 1).copy()  # [C,128,nb]

    dsq_cols = cols_of(dsqrt.astype(np.float32), 1.0)
    idsq_cols = cols_of(idsq.astype(np.float32), 1.0)

    r1 = np.zeros((N_CORES, ncols), dtype=np.float32)
    r1[:, :n_shard] = (-re * ds2).astype(np.float32).reshape(N_CORES, n_shard)
    r1_rep = np.broadcast_to(r1[:, None, :], (N_CORES, 64, ncols)).copy()

    # signal shards, p-major padded
    sig = np.zeros((N_CORES, ncols, F_IN), dtype=np.float32)
    sig[:, :n_shard] = np.asarray(signal, np.float32).reshape(N_CORES, n_shard, F_IN)
    sig_pm = sig.reshape(N_CORES, nb, P, F_IN).transpose(0, 2, 1, 3).reshape(
        N_CORES, ncols, F_IN
    ).copy()

    iota = np.broadcast_to(np.arange(P, dtype=np.float32), (P, P)).copy()
    ident = np.eye(P, dtype=np.float32)
    w_in = np.concatenate([W[:P, :], W[P:, :]], axis=1).astype(np.float32)  # [128,512]
    b_rep = np.broadcast_to(np.asarray(b, np.float32), (P, F_OUT)).copy()

    cfg = dict(
        n_nodes=n_nodes, n_shard=n_shard, nb=nb, ncols=ncols,
        tab_rows=tab_rows, half_rows=half_rows, T0=T0, T1=T1,
        nt_blk=nt_blk, nt=nt, ch=ch, chunk_plan=tuple(chunk_plan),
        c1=float(c1), c2=float(c2),
    )
    in_maps = []
    for c in range(N_CORES):
        in_maps.append({
            "sig": sig_pm[c].reshape(P, nb * F_IN),
            "idx0": idx_arrs[0][c],
            "idx1": idx_arrs[1][c],
            "dstv": dstv_arrs[c],
            "dsq": dsq_cols[c],
            "idsq": idsq_cols[c],
            "r1": r1_rep[c],
            "iota": iota,
            "ident": ident,
            "w_in": w_in,
            "b_rep": b_rep,
        })
    return cfg, in_maps


# ---------------------------------------------------------------------------
# Bass program
# ---------------------------------------------------------------------------
def build_program(cfg):
    nb = cfg["nb"]
    ncols = cfg["ncols"]
    T0, T1 = cfg["T0"], cfg["T1"]
    nt_blk = cfg["nt_blk"]
    nt = cfg["nt"]
    chunk_plan = cfg["chunk_plan"]
    half_rows = cfg["half_rows"]
    tab_rows = cfg["tab_rows"]
    c1, c2 = cfg["c1"], cfg["c2"]

    nc = bacc.Bacc(
        "TRN2", target_bir_lowering=False, debug=False,
        enable_asserts=False, num_devices=N_CORES,
    )

    sig_d = nc.dram_tensor("sig", [P, nb * F_IN], FP32, kind="ExternalInput")
    idx_d = [
        nc.dram_tensor("idx0", [P, nb * T0 * 8], I16, kind="ExternalInput"),
        nc.dram_tensor("idx1", [P, nb * T1 * 8], I16, kind="ExternalInput"),
    ]
    dstv_d = nc.dram_tensor("dstv", [P, nt], FP32, kind="ExternalInput")
    dsq_d = nc.dram_tensor("dsq", [P, nb], FP32, kind="ExternalInput")
    idsq_d = nc.dram_tensor("idsq", [P, nb], FP32, kind="ExternalInput")
    r1_d = nc.dram_tensor("r1", [64, ncols], FP32, kind="ExternalInput")
    iota_d = nc.dram_tensor("iota", [P, P], FP32, kind="ExternalInput")
    ident_d = nc.dram_tensor("ident", [P, P], FP32, kind="ExternalInput")
    w_d = nc.dram_tensor("w_in", [P, 2 * F_OUT], FP32, kind="ExternalInput")
    brep_d = nc.dram_tensor("b_rep", [P, F_OUT], FP32, kind="ExternalInput")
    out_d = nc.dram_tensor("out", [ncols, F_OUT], FP32, kind="ExternalOutput")

    rg = [list(range(N_CORES))]
    mult = mybir.AluOpType.mult
    add = mybir.AluOpType.add
    sub = mybir.AluOpType.subtract
    iseq = mybir.AluOpType.is_equal
    Relu = mybir.ActivationFunctionType.Relu
    Abs = mybir.ActivationFunctionType.Abs

    with tile.TileContext(nc) as tc:
        with (
            tc.tile_pool(name="const", bufs=1) as constp,
            tc.tile_pool(name="state", bufs=1) as statep,
            tc.tile_pool(name="yrows", bufs=2) as yrowsp,
            tc.tile_pool(name="chunk", bufs=3) as chunkp,
            tc.tile_pool(name="oh", bufs=6) as ohp,
            tc.tile_pool(name="work", bufs=3) as workp,
            tc.tile_pool(name="psA", bufs=3, space="PSUM") as psp,
            tc.tile_pool(name="psT", bufs=2, space="PSUM") as pstp,
            tc.tile_pool(name="psO", bufs=2, space="PSUM") as psop,
            tc.tile_pool(name="dram", bufs=2, space="DRAM") as dramp,
        ):
            # ---- constants into SBUF
            iota_t = constp.tile([P, P], FP32, tag="iota")
            nc.sync.dma_start(iota_t[:], iota_d[:])
            ident_t = constp.tile([P, P], FP32, tag="ident")
            nc.sync.dma_start(ident_t[:], ident_d[:])
            dstv_t = constp.tile([P, nt], FP32, tag="dstv")
            nc.sync.dma_start(dstv_t[:], dstv_d[:])
            dsq_t = constp.tile([P, nb], FP32, tag="dsq")
            nc.sync.dma_start(dsq_t[:], dsq_d[:])
            idsq_t = constp.tile([P, nb], FP32, tag="idsq")
            nc.sync.dma_start(idsq_t[:], idsq_d[:])
            r1_t = constp.tile([64, ncols], FP32, tag="r1")
            nc.sync.dma_start(r1_t[:], r1_d[:])
            w_t = constp.tile([P, 2 * F_OUT], FP32, tag="w")
            nc.sync.dma_start(w_t[:], w_d[:])
            brep_t = constp.tile([P, F_OUT], FP32, tag="brep")
            nc.sync.dma_start(brep_t[:], brep_d[:])
            idx_t = []
            for h, T in ((0, T0), (1, T1)):
                it = constp.tile([P, nb * T * 8], I16, tag=f"idx{h}", name=f"it{h}")
                nc.sync.dma_start(it[:], idx_d[h][:])
                idx_t.append(it)
            one_t = constp.tile([P, 1], FP32, tag="one")
            nc.gpsimd.memset(one_t[:], 1.0)
            two_t = constp.tile([P, 1], FP32, tag="two")
            nc.gpsimd.memset(two_t[:], 2.0)

            # ---- stacked states for the final matmul: yA=[Y0;Y1], yB=[Y2;Y3]
            # all elementwise compute happens on partitions 0..63; odd states
            # are staged into [64:128] via SBUF->SBUF DMA.
            yA = statep.tile([P, ncols], FP32, tag="yA")
            yB = statep.tile([P, ncols], FP32, tag="yB")
            y1lo = statep.tile([64, ncols], FP32, tag="y1lo")

            # ---- init: Y0 rows = sig * dsqrt (in place); Y0T via PE transpose
            y0r = yrowsp.tile([P, nb * F_IN], FP32, tag="sig")
            nc.sync.dma_start(y0r[:], sig_d[:])
            for t in range(nb):
                fs = slice(t * F_IN, (t + 1) * F_IN)
                nc.vector.tensor_scalar(
                    out=y0r[:, fs], in0=y0r[:, fs],
                    scalar1=dsq_t[:, t:t + 1], scalar2=None, op0=mult,
                )
                pst = pstp.tile([64, P], FP32, tag="tp")
                nc.tensor.transpose(pst[:], y0r[:, fs], ident_t[:])
                nc.vector.tensor_copy(
                    out=yA[0:64, t * P:(t + 1) * P], in_=pst[:]
                )
            ag_in0 = dramp.tile([P, nb * F_IN], FP32, tag="agin")
            nc.sync.dma_start(ag_in0[:], y0r[:])
            prev_ag_in = ag_in0

            act_stride = max(1, int(round(1.0 / ACT_FRAC))) if ACT_FRAC > 0 else 0

            # ---- Chebyshev iterations (all compute on partitions 0..63)
            for k in range(1, K_CHEB):
                table = dramp.tile([tab_rows, F_IN], FP32, tag="table")
                nc.gpsimd.collective_compute(
                    "AllGather", mybir.AluOpType.bypass, replica_groups=rg,
                    ins=[prev_ag_in[:].opt()], outs=[table[:].opt()],
                )

                chunks = [[], []]
                for h, T in ((0, T0), (1, T1)):
                    tab_half = table[h * half_rows:(h + 1) * half_rows, :]
                    for bk in range(nb):
                        ct = chunkp.tile(
                            [P, T, F_IN], FP32, tag=f"ch{h}", name=f"ct{h}"
                        )
                        for s in range(0, T, 4):
                            cw = min(4, T - s)
                            nc.gpsimd.dma_gather(
                                ct[:, s:s + cw, :],
                                tab_half,
                                idx_t[h][:, (bk * T + s) * 8:(bk * T + s + cw) * 8],
                                cw * P, cw * P, F_IN,
                            )
                        chunks[h].append(ct)

                scale2 = k >= 2
                ykr = None
                if k < K_CHEB - 1:
                    ykr = yrowsp.tile([P, nb * F_IN], FP32, tag="yrows", name="ykr")
                for bk in range(nb):
                    ps = psp.tile([64, P], FP32, tag="agg")
                    for j in range(nt_blk):
                        h = 0 if j < T0 else 1
                        jj = j if h == 0 else j - T0
                        lhs = chunks[h][bk][:, jj, :]
                        gt = bk * nt_blk + j
                        oh = ohp.tile([P, P], FP32, tag="oh")
                        if act_stride and gt % act_stride == 0:
                            tmp = ohp.tile([P, P], FP32, tag="ohtmp")
                            nc.scalar.activation(
                                tmp[:], iota_t[:], Abs,
                                bias=dstv_t[:, gt:gt + 1], scale=-1.0,
                            )
                            nc.scalar.activation(
                                oh[:], tmp[:], Relu,
                                bias=(two_t[:] if scale2 else one_t[:]),
                                scale=-2.0 if scale2 else -1.0,
                            )
                        else:
                            if scale2:
                                nc.vector.tensor_scalar(
                                    out=oh[:], in0=iota_t[:],
                                    scalar1=dstv_t[:, gt:gt + 1],
                                    scalar2=2.0, op0=iseq, op1=mult,
                                )
                            else:
                                nc.vector.tensor_scalar(
                                    out=oh[:], in0=iota_t[:],
                                    scalar1=dstv_t[:, gt:gt + 1],
                                    scalar2=None, op0=iseq,
                                )
                        nc.tensor.matmul(
                            out=ps[:], lhsT=lhs, rhs=oh[:],
                            start=(j == 0), stop=(j == nt_blk - 1),
                        )

                    # recurrence for this block, partitions 0..63
                    cs = slice(bk * P, (bk + 1) * P)
                    if k == 1:
                        dst_sl = y1lo[:, cs]
                    elif k == 2:
                        dst_sl = yB[0:64, cs]
                    else:
                        y3t = workp.tile([64, P], FP32, tag="y3t")
                        dst_sl = y3t[:]
                    if k == 1:
                        if c1 == 0.0:
                            nc.vector.tensor_tensor(
                                out=dst_sl, in0=ps[:], in1=r1_t[:, cs], op=mult
                            )
                        else:
                            u = workp.tile([64, P], FP32, tag="u")
                            nc.vector.tensor_tensor(
                                out=u[:], in0=ps[:], in1=r1_t[:, cs], op=mult
                            )
                            w_ = workp.tile([64, P], FP32, tag="wsc")
                            nc.vector.tensor_scalar(
                                out=w_[:], in0=yA[0:64, cs],
                                scalar1=c1, scalar2=None, op0=mult,
                            )
                            nc.vector.tensor_tensor(
                                out=dst_sl, in0=u[:], in1=w_[:], op=add
                            )
                    else:
                        prev_sl = y1lo[:, cs] if k == 2 else yB[0:64, cs]
                        pp_sl = yA[0:64, cs] if k == 2 else y1lo[:, cs]
                        u = workp.tile([64, P], FP32, tag="u")
                        nc.vector.tensor_tensor(
                            out=u[:], in0=ps[:], in1=r1_t[:, cs], op=mult
                        )
                        if c2 != 0.0:
                            w_ = workp.tile([64, P], FP32, tag="wsc")
                            nc.vector.tensor_scalar(
                                out=w_[:], in0=prev_sl,
                                scalar1=c2, scalar2=None, op0=mult,
                            )
                            u2 = workp.tile([64, P], FP32, tag="u2")
                            nc.vector.tensor_tensor(
                                out=u2[:], in0=u[:], in1=w_[:], op=add
                            )
                        else:
                            u2 = u
                        nc.vector.tensor_tensor(
                            out=dst_sl, in0=u2[:], in1=pp_sl, op=sub
                        )
                        if k == 3:
                            nc.sync.dma_start(yB[64:P, cs], dst_sl)

                    if k < K_CHEB - 1:
                        src_sl = y1lo[:, cs] if k == 1 else yB[0:64, cs]
                        pst = pstp.tile([P, F_IN], FP32, tag="tp")
                        nc.tensor.transpose(pst[:], src_sl, ident_t[:64, :64])
                        nc.vector.tensor_copy(
                            out=ykr[:, bk * F_IN:(bk + 1) * F_IN], in_=pst[:]
                        )

                if k == 1:
                    nc.sync.dma_start(yA[64:P, :], y1lo[:])
                if k < K_CHEB - 1:
                    ag_in = dramp.tile([P, nb * F_IN], FP32, tag="agin")
                    nc.sync.dma_start(ag_in[:], ykr[:])
                    prev_ag_in = ag_in

            # ---- final: out = relu(idsq * (Xt @ W) + b)
            for t in range(nb):
                cs = slice(t * P, (t + 1) * P)
                pso = psop.tile([P, F_OUT], FP32, tag="po")
                nc.tensor.matmul(
                    out=pso[:], lhsT=yA[:, cs], rhs=w_t[:, :F_OUT],
                    start=True, stop=False,
                )
                nc.tensor.matmul(
                    out=pso[:], lhsT=yB[:, cs], rhs=w_t[:, F_OUT:],
                    start=False, stop=True,
                )
                u = workp.tile([P, F_OUT], FP32, tag="fo")
                nc.vector.tensor_scalar(
                    out=u[:], in0=pso[:], scalar1=idsq_t[:, t:t + 1],
                    scalar2=None, op0=mult,
                )
                v = workp.tile([P, F_OUT], FP32, tag="fo2")
                nc.vector.tensor_tensor(out=v[:], in0=u[:], in1=brep_t[:], op=add)
                r_ = workp.tile([P, F_OUT], FP32, tag="fo3")
                nc.scalar.activation(r_[:], v[:], Relu)
                nc.sync.dma_start(out_d[t * P:(t + 1) * P, :], r_[:])

    nc.compile()
    return nc


# ---------------------------------------------------------------------------
# entry point
# ---------------------------------------------------------------------------
_CACHE = {}


def _run(signal, src, dst, lambda_max, W, b, trace=False):
    cfg, in_maps = preprocess(signal, src, dst, lambda_max, W, b)
    key = (cfg["T0"], cfg["T1"], cfg["c1"], cfg["c2"], cfg["nb"])
    if key not in _CACHE:
        _CACHE[key] = build_program(cfg)
    nc = _CACHE[key]
    res = run_bass_kernel_spmd(
        nc, in_maps, core_ids=list(range(N_CORES)), trace=trace
    )
    n_shard = cfg["n_shard"]
    outs = [res.results[c]["out"][:n_shard] for c in range(N_CORES)]
    full = np.concatenate(outs, axis=0)[:cfg["n_nodes"]]
    return full, res


def kernel(signal, src, dst, lambda_max, W, b):
    signal = np.asarray(signal, np.float32)
    src = np.asarray(src, np.int32)
    dst = np.asarray(dst, np.int32)
    lambda_max = np.asarray(lambda_max, np.float32)
    W = np.asarray(W, np.float32)
    b = np.asarray(b, np.float32)
    out, _ = _run(signal, src, dst, lambda_max, W, b, trace=False)
    return out

